# revision 19
# baseline (speedup 1.0000x reference)
"""Trainium2 Bass kernel for nn_CustomLoss: weighted-CE + all-pairs windowed SSIM BCE loss.

Strategy: pure data-parallel over batch B=32 -> 4 videos per core on 8 cores.
Per core, for each video:
  - load features [8f,16c,64,64] with h on partitions (2 channels stacked -> 128 partitions)
  - W-filter (7-tap box along w) for per-frame x and x^2 on DVE (shift-add tree, bf16)
  - H-filter via TensorE matmul with a banded 0/1 block-diagonal matrix -> ux, uxx
  - pair products t = x_i*x_j on DVE; fused 49-tap (7 H-band matmuls x 7 shifted w offsets,
    PSUM-accumulated) -> uxy
  - SSIM map algebra on DVE/ACT (bf16), reciprocal via exp(-ln(den)) on ScalarE
  - per-pair spatial reduction via ones-vector matmul on TensorE -> DMA partial sums out
Host: tiny tail (28 pairs x 464 partial sums -> ssim means -> BCE; CE from predictions).
"""

import numpy as np
import ml_dtypes

B, F, C, H, W = 32, 8, 16, 64, 64
NCORES = 8
BSH = B // NCORES          # 4 videos per core
CP = C // 2                # channel pairs stacked on partitions
WIN = 7
HO = H - WIN + 1           # 58
NP_WIN = WIN * WIN
COV_NORM = NP_WIN / (NP_WIN - 1.0)
C1 = 0.01 ** 2
C2 = 0.03 ** 2
NPAIR = F * (F - 1) // 2   # 28

_CACHE = {}


def _pair_index(i, j):
    # triu order (row-major), matches np.triu_indices(F, 1)
    base = i * (2 * F - i - 1) // 2
    return base + (j - i - 1)


def _build_program():
    import concourse.bass as bass
    import concourse.bacc as bacc
    import concourse.tile as tile
    from concourse import mybir

    f32 = mybir.dt.float32
    bf16 = mybir.dt.bfloat16
    AF = mybir.ActivationFunctionType

    nc = bacc.Bacc(None, target_bir_lowering=False)

    feat = nc.dram_tensor([BSH, F, C, H, W], f32, kind="ExternalInput")
    band = nc.dram_tensor([128, 2 * HO], bf16, kind="ExternalInput")
    onesw = nc.dram_tensor([128, 1], bf16, kind="ExternalInput")
    out = nc.dram_tensor([BSH, NPAIR, CP * HO], bf16, kind="ExternalOutput")

    # element strides of feat
    s_b = F * C * H * W
    s_f = C * H * W
    s_c = H * W

    def ap_of(x):
        return x[:] if not isinstance(x, bass.AP) else x

    with tile.TileContext(nc) as tc:
        with (
            tc.tile_pool(name="consts", bufs=1) as consts,
            tc.tile_pool(name="stage", bufs=1) as stage_p,
            tc.tile_pool(name="xp", bufs=2) as xp,
            tc.tile_pool(name="frameq", bufs=2) as frameq,
            tc.tile_pool(name="pairp", bufs=3) as pairp,
            tc.tile_pool(name="math", bufs=7) as mathp,
            tc.tile_pool(name="psum_mm", bufs=3, space="PSUM") as psum_mm,
            tc.tile_pool(name="psum_red", bufs=1, space="PSUM") as psum_red,
            tc.tile_pool(name="obuf_p", bufs=2) as obuf_p,
        ):
            band_sb = consts.tile([128, 2 * HO], bf16)
            nc.sync.dma_start(out=band_sb[:], in_=band[:])
            ones_sb = consts.tile([128, 1], bf16)
            nc.sync.dma_start(out=ones_sb[:], in_=onesw[:])

            def bcast_j(t, i, nj):
                # t is a tile [128, F, CP, X]; return AP [128, nj, CP, X] broadcasting f=i
                base = t[:, i, :, :]
                return bass.AP(
                    tensor=base.tensor,
                    offset=base.offset,
                    ap=[base.ap[0], [0, nj]] + list(base.ap[1:]),
                )

            for b in range(BSH):
                stg = stage_p.tile([128, F, CP, W], f32, tag="stg")
                for q in range(2):
                    src = ap_of(feat)
                    src_ap = bass.AP(
                        tensor=src.tensor,
                        offset=src.offset + b * s_b + q * s_c,
                        ap=[[W, H], [s_f, F], [2 * s_c, CP], [1, W]],
                    )
                    nc.sync.dma_start(out=stg[64 * q:64 * q + 64, :, :, :], in_=src_ap)

                X = xp.tile([128, F, CP, W], bf16, tag="X")
                nc.scalar.activation(X[:], stg[:], AF.Copy)
                X2 = xp.tile([128, F, CP, W], bf16, tag="X2")
                nc.vector.tensor_mul(X2[:], X[:], X[:])

                ux = frameq.tile([128, F, CP, HO], bf16, tag="ux")
                uxx = frameq.tile([128, F, CP, HO], bf16, tag="uxx")
                for kf in range(F):
                    for src_t, dst_t in ((X, ux), (X2, uxx)):
                        ps = psum_mm.tile([128, CP * HO], f32, tag="ps")
                        for dw in range(WIN):
                            nc.tensor.matmul(
                                ps[0:116, :],
                                band_sb[:, 0:116],
                                src_t[:, kf, :, dw:dw + HO],
                                start=(dw == 0),
                                stop=(dw == WIN - 1),
                            )
                        nc.scalar.activation(
                            dst_t[0:116, kf, :, :], ps[0:116, :], AF.Copy,
                            scale=1.0 / NP_WIN,
                        )

                A0 = mathp.tile([128, F, CP, HO], bf16, tag="mt")
                nc.vector.tensor_mul(A0[0:116], ux[0:116], ux[0:116])
                A1 = frameq.tile([128, F, CP, HO], bf16, tag="A1")
                nc.vector.tensor_scalar_add(A1[0:116], A0[0:116], C1 / 2.0)
                V0 = mathp.tile([128, F, CP, HO], bf16, tag="mt")
                nc.vector.tensor_sub(V0[0:116], uxx[0:116], A0[0:116])
                V1 = frameq.tile([128, F, CP, HO], bf16, tag="V1")
                nc.vector.tensor_scalar(
                    V1[0:116], V0[0:116], COV_NORM, C2 / 2.0,
                    mybir.AluOpType.mult, mybir.AluOpType.add,
                )

                for i in range(F - 1):
                    nj = F - 1 - i
                    t = pairp.tile([128, nj, CP, W], bf16, tag="t")
                    nc.vector.tensor_mul(t[:], bcast_j(X, i, nj), X[:, i + 1:F, :, :])

                    uxy = pairp.tile([128, nj, CP, HO], bf16, tag="uxy")
                    for j in range(nj):
                        ps = psum_mm.tile([128, CP * HO], f32, tag="ps")
                        for dw in range(WIN):
                            nc.tensor.matmul(
                                ps[0:116, :],
                                band_sb[:, 0:116],
                                t[:, j, :, dw:dw + HO],
                                start=(dw == 0),
                                stop=(dw == WIN - 1),
                            )
                        nc.scalar.activation(
                            uxy[0:116, j, :, :], ps[0:116, :], AF.Copy,
                            scale=1.0 / NP_WIN,
                        )

                    m = mathp.tile([128, nj, CP, HO], bf16, tag="mt")
                    nc.vector.tensor_mul(
                        m[0:116], bcast_j(ux, i, nj)[0:116], ux[0:116, i + 1:F, :, :]
                    )
                    w_ = mathp.tile([128, nj, CP, HO], bf16, tag="mt")
                    nc.vector.tensor_sub(w_[0:116], uxy[0:116], m[0:116])
                    num1 = mathp.tile([128, nj, CP, HO], bf16, tag="mt")
                    nc.vector.tensor_scalar(
                        num1[0:116], m[0:116], 2.0, C1,
                        mybir.AluOpType.mult, mybir.AluOpType.add,
                    )
                    num2 = mathp.tile([128, nj, CP, HO], bf16, tag="mt")
                    nc.vector.tensor_scalar(
                        num2[0:116], w_[0:116], 2.0 * COV_NORM, C2,
                        mybir.AluOpType.mult, mybir.AluOpType.add,
                    )
                    num = mathp.tile([128, nj, CP, HO], bf16, tag="mt")
                    nc.vector.tensor_mul(num[0:116], num1[0:116], num2[0:116])
                    den1 = mathp.tile([128, nj, CP, HO], bf16, tag="mt")
                    nc.vector.tensor_add(
                        den1[0:116], bcast_j(A1, i, nj)[0:116], A1[0:116, i + 1:F, :, :]
                    )
                    den2 = mathp.tile([128, nj, CP, HO], bf16, tag="mt")
                    nc.vector.tensor_add(
                        den2[0:116], bcast_j(V1, i, nj)[0:116], V1[0:116, i + 1:F, :, :]
                    )
                    den = mathp.tile([128, nj, CP, HO], bf16, tag="mt")
                    nc.vector.tensor_mul(den[0:116], den1[0:116], den2[0:116])
                    # 1/den as Abs_reciprocal_sqrt(den)^2 -- both funcs share one
                    # ACT table set with Copy, so no table reloads
                    rsq = mathp.tile([128, nj, CP, HO], bf16, tag="mt")
                    nc.scalar.activation(rsq[0:116], den[0:116], AF.Abs_reciprocal_sqrt)
                    r2 = mathp.tile([128, nj, CP, HO], bf16, tag="mt")
                    nc.scalar.activation(r2[0:116], rsq[0:116], AF.Square)
                    S = mathp.tile([128, nj, CP, HO], bf16, tag="mt")
                    nc.vector.tensor_mul(S[0:116], num[0:116], r2[0:116])

                    # per-pair spatial sums: ones-matmul into 512-strided PSUM
                    # banks, then one strided copy PSUM->SBUF per group
                    j0 = 0
                    while j0 < nj:
                        gn = min(4, nj - j0)
                        psr = psum_red.tile([1, 4 * 512], f32, tag="psr")
                        for g in range(gn):
                            nc.tensor.matmul(
                                psr[0:1, g * 512:g * 512 + CP * HO],
                                ones_sb[0:116, 0:1],
                                S[0:116, j0 + g, :, :],
                            )
                        src = bass.AP(
                            tensor=psr[:].tensor,
                            offset=psr[:].offset,
                            ap=[psr[:].ap[0], [512, gn], [1, CP * HO]],
                        )
                        gbuf = obuf_p.tile([1, 4, CP * HO], bf16, tag="gbuf")
                        nc.scalar.activation(gbuf[:, 0:gn, :], src, AF.Copy)
                        p = _pair_index(i, i + 1 + j0)
                        nc.sync.dma_start(
                            out=out[b, p:p + gn, :], in_=gbuf[:, 0:gn, :]
                        )
                        j0 += gn

    nc.compile()
    return nc, feat.name, band.name, onesw.name, out.name


def _make_consts():
    band = np.zeros((128, 2 * HO), dtype=np.float32)
    for s in range(2):
        for ho in range(HO):
            band[64 * s + ho:64 * s + ho + WIN, HO * s + ho] = 1.0
    ones = np.zeros((128, 1), dtype=np.float32)
    ones[0:116] = 1.0
    return band.astype(ml_dtypes.bfloat16), ones.astype(ml_dtypes.bfloat16)


def kernel(predictions, features, labels):
    from concourse.bass_utils import run_bass_kernel_spmd

    if "prog" not in _CACHE:
        _CACHE["prog"] = _build_program()
    nc, feat_name, band_name, ones_name, out_name = _CACHE["prog"]

    band, ones = _make_consts()
    feats = np.ascontiguousarray(features, dtype=np.float32)
    in_maps = [
        {
            feat_name: feats[k * BSH:(k + 1) * BSH],
            band_name: band,
            ones_name: ones,
        }
        for k in range(NCORES)
    ]
    res = run_bass_kernel_spmd(nc, in_maps, core_ids=list(range(NCORES)))
    sums = np.concatenate([r[out_name] for r in res.results], axis=0)  # [32, 28, CP*HO]

    ssim_pair = sums.astype(np.float64).sum(-1) / (C * HO * HO)  # [32, 28]

    labels = np.asarray(labels).astype(np.int64)
    preds = np.asarray(predictions).astype(np.float64)

    # weighted CE (torch CrossEntropyLoss with weights [10, 1])
    mx = preds.max(axis=1, keepdims=True)
    logp = preds - mx - np.log(np.exp(preds - mx).sum(axis=1, keepdims=True))
    nll = -logp[np.arange(B), labels]
    wts = np.where(labels == 0, 10.0, 1.0)
    cce = (wts * nll).sum() / wts.sum()

    # BCE on mean pair-similarity
    sim = np.clip(ssim_pair + 0.5, 0.0, 1.0)
    avg_sim = sim.mean(axis=1)
    t = (labels == 0).astype(np.float64)
    log_p = np.maximum(np.log(np.maximum(avg_sim, 1e-300)), -100.0)
    log_1mp = np.maximum(np.log(np.maximum(1.0 - avg_sim, 1e-300)), -100.0)
    bce = -(t * log_p + (1.0 - t) * log_1mp)
    inconsistency = bce.mean()

    return np.float32(cce + 4.0 * inconsistency)

